# revision 4
# baseline (speedup 1.0000x reference)
"""CRF Viterbi decode (B=1024, T=1024, N=32) on 8 TRN2 NeuronCores.

Data-parallel: batch is split 128-per-core; the tiny [32,32] transition
matrix (and derived constant tables) is replicated to every core.

Per core the kernel runs entirely on the vector engine (DVE):
  forward  : scores = state+T' (one 1024-wide add), segmented reduce_max over
             prev, +logit, plus a per-batch "snapshot" of the state at
             t == len-1 (exact replacement for the reference's freeze).
             The full state history (T x 128 x 32 f32 = 128KB/partition)
             stays resident in SBUF.
  backtrace: two custom DVE ops per step recompute the argmax backpointer
             for just the needed tag: a page-masked max over the 32-column
             block selected by the current tag, then a first-argmax via an
             equality scan with a position encode.  The prev axis is laid
             out reversed so MAX-accum tie-breaking matches jnp.argmax's
             first-index semantics bit-exactly.
All f32 sums/maxes are performed in the same order as the reference, so the
output matches the jax reference exactly (including f32 ties).
"""
import sys
sys.path.insert(0, "/opt/trn_rl_repo")
from contextlib import ExitStack

import numpy as np

import concourse.bass as bass
import concourse.bacc as bacc
import concourse.mybir as mybir
import concourse.tile as tile
from concourse.bass_utils import run_bass_kernel_spmd

F32 = mybir.dt.float32
I32 = mybir.dt.int32
I8 = mybir.dt.int8
AX = mybir.AxisListType
OP = mybir.AluOpType

B, T, N = 1024, 1024, 32
PB = 128
NCORES = 8

_ops_cache = {}
_nc_cache = {}
_last_exec_ns = [None]


def register_custom_ops():
    if _ops_cache:
        return _ops_cache["BWMAX"], _ops_cache["BWARG"]
    from concourse.dve_spec import (
        Spec, Src0, Src1, C0, C1, C2, MaxNeg, eq, select, AluOp, lower, SubIdx, Idx,
    )
    from concourse.dve_ops import DveOp, OPS, has_src1
    from concourse.dve_uop import DveOpSpec
    import concourse.dve_ops as dom

    def make(name, spec, subdim):
        for o in OPS:
            if o.name == name:
                return o
        shas = {}
        for ver in ("v3", "v4"):
            s = DveOpSpec(name=name, opcode=0, uops=lower(spec, ver=ver),
                          rd1_en=has_src1(spec))
            shas[ver] = s.sha(ver)
        op = DveOp(name, spec, subdim=subdim, uops_sha=shas)
        OPS.append(op)
        dom.CUSTOM_DVE_SPECS[name] = spec
        dom._SUB_OPCODE_FOR_NAME[name] = dom._CUSTOM_DVE_ROW_BASE + len(OPS) - 1
        assert dom._SUB_OPCODE_FOR_NAME[name] < 0x20
        return op

    FMAX = np.float32(3.4028235e38)

    def ref1(in0, in1, c0, c1, c2):
        P, S, Nn = in0.shape
        pm = (np.arange(S)[None, :, None] == c0[:, :, None])
        x = (in0 + in1).astype(np.float32)
        xm = np.where(pm, x, -FMAX).astype(np.float32)
        return xm, xm.max(axis=(1, 2)).reshape(P, 1)

    def ref2(in0, in1, c0, c1, c2):
        P, K = in0.shape
        e = (in0 == c0).astype(np.float32)
        enc = (np.arange(K)[None, :] + c1 * c2).astype(np.float32)
        m = (e * enc).astype(np.float32)
        return m, m.max(axis=1, initial=-FMAX).reshape(P, 1)

    spec1 = Spec(body=select(eq(SubIdx, C0), Src0 + Src1, MaxNeg), accum=AluOp.MAX,
                 reference=ref1)
    spec2 = Spec(body=eq(Src0, C0) * (Idx + C1 * C2), accum=AluOp.MAX,
                 reference=ref2)

    from concourse.dve_spec import scan

    def ref3(in0, in1, c0, c1, c2):
        P, K = in0.shape
        x = (in0 + in1).astype(np.float32)
        r = np.maximum.accumulate(x, axis=1)
        m = ((x == r).astype(np.float32) * np.arange(K, dtype=np.float32)[None, :])
        return m, m.max(axis=1, initial=-FMAX).reshape(P, 1)

    _x3 = Src0 + Src1
    spec3 = Spec(body=eq(_x3, scan(AluOp.MAX, _x3)) * Idx, accum=AluOp.MAX,
                 reference=ref3)

    op1 = make("CRF_BWMAX", spec1, subdim=True)
    op2 = make("CRF_BWARG", spec2, subdim=False)
    op3 = make("CRF_BT32", spec3, subdim=False)
    _ops_cache["BWMAX"] = op1
    _ops_cache["BWARG"] = op2
    _ops_cache["BT32"] = op3
    return op1, op2


def build_nc(Tn, CH=64):
    if Tn < CH:
        CH = Tn
    op1, op2 = register_custom_ops()
    op3 = _ops_cache["BT32"]

    nc = bacc.Bacc("TRN2", target_bir_lowering=False, debug=False,
                   num_devices=NCORES)

    logits = nc.dram_tensor("logits", [PB, Tn, N], F32, kind="ExternalInput")
    trep_d = nc.dram_tensor("trep", [PB, N * N], F32, kind="ExternalInput")
    trev_d = nc.dram_tensor("trev", [PB, N * N], F32, kind="ExternalInput")
    meq_d = nc.dram_tensor("meq", [PB, Tn], I8, kind="ExternalInput")
    mlt_d = nc.dram_tensor("mlt", [PB, Tn], F32, kind="ExternalInput")
    mlt8_d = nc.dram_tensor("mlt8", [PB, Tn], I8, kind="ExternalInput")
    irev_d = nc.dram_tensor("irev", [PB, N], F32, kind="ExternalInput")
    iota_d = nc.dram_tensor("iota32", [PB, N], F32, kind="ExternalInput")
    trevw_d = nc.dram_tensor("trevw", [N, N], F32, kind="ExternalInput")
    out_d = nc.dram_tensor("out", [PB, Tn], I32, kind="ExternalOutput")

    with tile.TileContext(nc) as tc:
        with (
            tc.tile_pool(name="consts", bufs=1) as cpool,
            tc.tile_pool(name="states", bufs=1) as spool,
            tc.tile_pool(name="big", bufs=1) as bpool,
            tc.tile_pool(name="lchunks", bufs=2) as lpool,
            tc.tile_pool(name="small", bufs=1) as mpool,
            tc.tile_pool(name="psum", bufs=1, space="PSUM") as ppool,
        ):
            trep = cpool.tile([PB, N * N], F32, tag="trep")
            trev = cpool.tile([PB, N * N], F32, tag="trev")
            meq = cpool.tile([PB, Tn], I8, tag="meq")
            mlt = cpool.tile([PB, Tn], F32, tag="mlt")
            mlt8 = cpool.tile([PB, Tn], I8, tag="mlt8")
            irev = cpool.tile([PB, N], F32, tag="irev")
            iota32 = cpool.tile([PB, N], F32, tag="iota32")
            trevw = cpool.tile([N, N], F32, tag="trevw")
            states = spool.tile([PB, Tn * N], F32, tag="states")
            scores = bpool.tile([PB, N * N], F32, tag="scores")
            junk = bpool.tile([PB, N * N], F32, tag="junk")
            tags = bpool.tile([PB, Tn], F32, tag="tags")
            outi = bpool.tile([PB, Tn], I32, tag="outi")
            snap = mpool.tile([PB, N], F32, tag="snap")
            eqs = mpool.tile([PB, N], F32, tag="eqs")
            red = mpool.tile([PB, N], F32, tag="red")
            m1 = mpool.tile([PB, 1], F32, tag="m1")
            a1 = mpool.tile([PB, 1], F32, tag="a1")
            pv = mpool.tile([PB, 1], F32, tag="pv")
            onehot = mpool.tile([PB, N], F32, tag="onehot")
            onehotT = mpool.tile([PB, N], F32, tag="onehotT")
            tsel = ppool.tile([PB, N], F32, tag="tsel")

            nc.vector.memset(snap[:], 0.0)
            nc.sync.dma_start(out=trep[:], in_=trep_d.ap())
            nc.sync.dma_start(out=trev[:], in_=trev_d.ap())
            nc.sync.dma_start(out=meq[:], in_=meq_d.ap())
            nc.sync.dma_start(out=mlt[:], in_=mlt_d.ap())
            nc.sync.dma_start(out=mlt8[:], in_=mlt8_d.ap())
            nc.sync.dma_start(out=irev[:], in_=irev_d.ap())
            nc.sync.dma_start(out=iota32[:], in_=iota_d.ap())
            nc.sync.dma_start(out=trevw[:], in_=trevw_d.ap())

            trep3 = trep[:].rearrange("p (c v) -> p c v", v=N)
            trev3 = trev[:].rearrange("p (c v) -> p c v", v=N)
            scores3 = scores[:].rearrange("p (c v) -> p c v", v=N)

            nchunks = (Tn + CH - 1) // CH
            for c in range(nchunks):
                lt = lpool.tile([PB, CH * N], F32, tag="lchunk")
                nc.sync.dma_start(
                    out=lt[:].rearrange("p (t v) -> p t v", v=N),
                    in_=logits.ap()[:, c * CH:(c + 1) * CH, :],
                )
                if c == 0:
                    nc.vector.tensor_copy(out=states[:, 0:N], in_=lt[:, 0:N])
                    nc.vector.copy_predicated(
                        out=snap[:],
                        mask=meq[:, 0:1].to_broadcast((PB, N)),
                        data=states[:, 0:N],
                    )
                for i in range(CH):
                    t = c * CH + i
                    if t == 0:
                        continue
                    sprev = states[:, (t - 1) * N: t * N]
                    scur = states[:, t * N: (t + 1) * N]
                    sprev_b = sprev.rearrange("p (o v) -> p o v", o=1) \
                                   .to_broadcast((PB, N, N))
                    nc.vector.tensor_tensor(
                        out=scores3, in0=sprev_b, in1=trep3, op=OP.add)
                    nc.vector.tensor_reduce(
                        out=red[:], in_=scores3, axis=AX.X, op=OP.max)
                    nc.vector.tensor_tensor(
                        out=scur, in0=red[:], in1=lt[:, i * N:(i + 1) * N],
                        op=OP.add)
                    nc.vector.copy_predicated(
                        out=snap[:],
                        mask=meq[:, t:t + 1].to_broadcast((PB, N)),
                        data=scur,
                    )

            nc.vector.tensor_reduce(out=m1[:], in_=snap[:], axis=AX.X, op=OP.max)
            nc.vector.tensor_tensor(
                out=eqs[:], in0=snap[:], in1=m1[:].to_broadcast((PB, N)),
                op=OP.is_equal)
            nc.vector.tensor_tensor(out=red[:], in0=eqs[:], in1=irev[:],
                                    op=OP.mult)
            nc.vector.tensor_reduce(out=a1[:], in_=red[:], axis=AX.X, op=OP.max)
            nc.vector.tensor_scalar(
                out=tags[:, Tn - 1:Tn], in0=a1[:], scalar1=-1.0, scalar2=31.0,
                op0=OP.mult, op1=OP.add)

            with ExitStack() as ctx:
                for t in range(Tn - 2, -1, -1):
                    st = states[:, t * N: (t + 1) * N]
                    tagcol = tags[:, t + 1:t + 2]
                    nc.vector.tensor_scalar(
                        out=onehot[:], in0=iota32[:], scalar1=tagcol,
                        scalar2=None, op0=OP.is_equal)
                    nc.vector.transpose(out=onehotT[:], in_=onehot[:])
                    for blk in range(4):
                        nc.tensor.matmul(
                            ctx, out=tsel[blk * N:(blk + 1) * N, :],
                            lhsT=onehotT[blk * N:(blk + 1) * N, :],
                            rhs=trevw[:], start=True, stop=True)
                    nc.vector._custom_dve(
                        op3, out=red[:], in0=tsel[:], in1=st[:, ::-1],
                        accum_out=a1[:])
                    nc.vector.tensor_scalar(
                        out=pv[:], in0=a1[:], scalar1=-1.0, scalar2=31.0,
                        op0=OP.mult, op1=OP.add)
                    nc.vector.tensor_copy(out=tags[:, t:t + 1], in_=tagcol)
                    nc.vector.copy_predicated(
                        out=tags[:, t:t + 1], mask=mlt8[:, t + 1:t + 2], data=pv[:])

            nc.vector.tensor_tensor(out=tags[:], in0=tags[:], in1=mlt[:],
                                    op=OP.mult)
            nc.vector.tensor_copy(out=outi[:], in_=tags[:])
            nc.sync.dma_start(out=out_d.ap(), in_=outi[:])

    nc.compile()
    return nc


def make_inputs_for_core(logits_shard, lens_shard, Tn, Tmat):
    trep = np.ascontiguousarray(Tmat.T).reshape(1, N * N)
    trev = np.ascontiguousarray(Tmat[::-1, :].T).reshape(1, N * N)
    tcol = np.arange(Tn)[None, :]
    lens = lens_shard.astype(np.int64)[:, None]
    meq = (lens == (tcol + 1)).astype(np.float32)
    mlt = (tcol < lens).astype(np.float32)
    irev = (31.0 - np.arange(N, dtype=np.float32))[None, :]
    rep = lambda a: np.ascontiguousarray(
        np.broadcast_to(a, (PB, a.shape[1])), dtype=np.float32)
    return {
        "logits": np.ascontiguousarray(logits_shard, dtype=np.float32),
        "trep": rep(trep),
        "trev": rep(trev),
        "meq": np.ascontiguousarray(meq, dtype=np.int8),
        "mlt": np.ascontiguousarray(mlt, dtype=np.float32),
        "mlt8": np.ascontiguousarray(mlt, dtype=np.int8),
        "irev": rep(irev),
        "iota32": rep(np.arange(N, dtype=np.float32)[None, :]),
        "trevw": np.ascontiguousarray(Tmat[::-1, :].T, dtype=np.float32),
    }


def last_exec_time_ns():
    return _last_exec_ns[0]


def kernel(logits, transitions, sequence_lengths, _trace=False):
    logits = np.asarray(logits, dtype=np.float32)
    Tmat = np.asarray(transitions, dtype=np.float32)
    lens = np.asarray(sequence_lengths)
    Bn, Tn, Nn = logits.shape
    assert Nn == N and Bn % NCORES == 0

    if Tn not in _nc_cache:
        _nc_cache[Tn] = build_nc(Tn)
    nc = _nc_cache[Tn]

    in_maps = []
    for i in range(NCORES):
        sl = slice(i * PB, (i + 1) * PB)
        in_maps.append(make_inputs_for_core(logits[sl], lens[sl], Tn, Tmat))

    kw = {}
    if _trace:
        kw = dict(trace=True, trace_cores=[0])
    res = run_bass_kernel_spmd(nc, in_maps, core_ids=list(range(NCORES)), **kw)
    _last_exec_ns[0] = getattr(res, "exec_time_ns", None)

    out = np.concatenate([res.results[i]["out"] for i in range(NCORES)], axis=0)
    return out.astype(np.int32)
